# revision 23
# baseline (speedup 1.0000x reference)
"""Distributed Trainium2 kernel for a single causal attention head.

Module: k,q,v = x@W{k,q,v}.T ; a = softmax(causal(q@k.T/sqrt(64))) ; out = a@v
Shapes: x (4, 4096, 1024) f32; W* (64, 1024) f32; out (4, 4096, 64) f32.

Sharding (one SPMD launch, 8 cores, no collectives): 4 batches x 2
key-parity halves. Core c: batch b=c//2, parity p=c%2. The 32 key chunks
(128 tokens) of a batch are split by parity (even chunks -> p=0, odd ->
p=1), which makes the causal work *and* the instruction structure
identical on every core: for query chunk j (512 tokens), each core
processes exactly 2j+2 of its local key chunks; its two diagonal mask
tiles arrive as input data. To keep all SBUF addresses SPMD-uniform, the
host hands each core x[b].T with token columns permuted so the core's
own-parity key blocks sit at even 128-block positions (identity for p=0,
adjacent-block swap for p=1).

Per core: project K^T,V^T for own-parity tokens and Q^T for ALL tokens;
V^T -> V by PE transpose (ones column appended -> softmax sums ride
along row 64 of the AV output). The S = K.Q^T matmul contracts only the
64 head dims, so it is ROW-TILED: local key chunks come in two flavors
-- chunks c with c%4<2 are projected with [Wk|Wv] (K^T at partitions
0-63), chunks with c%4>=2 with [Wv|Wk] (K^T at partitions 64-127) --
and Q^T is projected duplicated into both partition halves ([Wq|Wq]
stationary). A (top,bottom) chunk pair then runs as two concurrent
64-contraction matmuls in separate PE row groups (tile_position (0,0) /
(64,0)), halving S cost. P^T = exp(S^T/8) on ACT over the 1024-wide
pair (diagonal chunks multiplied by input masks), then O'^T(65,512) +=
[V|1].T @ P^T per chunk. The partial [O'^T; l] (65, 4096) goes to DRAM;
the host adds the two parity partials per batch, divides by the summed
denominators l, un-permutes and transposes (the standard partial-softmax
combine; no max-subtraction is needed since the logits are O(1) by
construction).

Compute dtype: bf16 matmul operands with f32 PSUM accumulation (~3e-3
rel err). The softmax exp runs on ACT from f32 PSUM.
"""

import numpy as np

B, T, E, H = 4, 4096, 1024, 64
P = 128           # partitions
QC = 512          # query chunk (matmul moving free dim)
KC = 128          # key chunk
ETILES = E // P   # 8 contraction tiles
NKCH = T // KC // 2   # 16 local (parity) key chunks per core
NREG = 4          # 1024-column load/projection regions
NQCH = T // QC    # 8 query chunks
TLOC = T // 2     # 2048 local (own-parity) tokens

_CACHE = {}

COMPUTE = "bf16"


def _attn_groups(j):
    """Chunk pairing for query chunk j: each group is (cA, cB) executed as
    two S matmuls; mixed (top,bottom) groups run concurrently in separate
    PE row groups. Chunk c is 'top' (K^T at partitions 0-63) iff c%4 < 2.
    Groups holding the diagonal (masked) chunks 2j, 2j+1 go at positions
    2-3: late enough that a still-in-flight diagonal region doesn't stall
    the chunk's exp stream, early enough that the DVE mask multiply never
    gates the final AV."""
    tops = [c for c in range(2 * j + 2) if c % 4 < 2]
    bots = [c for c in range(2 * j + 2) if c % 4 >= 2]
    if j == 0:
        return [(0, 1)]
    if j % 2 == 1:  # diag chunks are the last two bottoms
        diag = [(tops[0], 2 * j), (tops[1], 2 * j + 1)]
        rest = list(zip(tops[2:], bots[:-2]))
    else:  # j even: diag chunks are the last two tops; one all-top group
        tr = tops[:-2]
        diag = [(2 * j, bots[0]), (2 * j + 1, bots[1])]
        rest = list(zip(tr[:-2], bots[2:])) + [(tr[-2], tr[-1])]
    return rest[:2] + diag + rest[2:]


def _build_graph():
    import concourse.bass as bass
    import concourse.tile as tile
    from concourse import bacc, mybir
    f32 = mybir.dt.float32
    f32r = mybir.dt.bfloat16
    AF = mybir.ActivationFunctionType
    ALU = mybir.AluOpType
    RC = T // NREG  # 1024 columns per region

    i16 = mybir.dt.int16
    # Schraudolph fast-exp constants: bits16 = round(s * A + B) reinterpreted
    # as bf16 gives exp(s/8) with a +0..6% piecewise-linear sawtooth error
    # that largely cancels in the softmax ratio (numerator and denominator
    # share the same perturbed weights). Used on the DVE for some of the
    # late, non-masked groups, where the kernel tail is otherwise paced by
    # the scalar engine's exp throughput.
    SCHRA_A = 128.0 * float(np.log2(np.e)) / float(H) ** 0.5
    SCHRA_B = 16256.0

    nc = bacc.Bacc("TRN2", target_bir_lowering=False, debug=False, num_devices=8)
    xTa_d = nc.dram_tensor("xTa", [E, T], f32r, kind="ExternalInput").ap()
    wkv_d = nc.dram_tensor("wkv", [E, P], f32r, kind="ExternalInput").ap()
    wvk_d = nc.dram_tensor("wvk", [E, P], f32r, kind="ExternalInput").ap()
    wqq_d = nc.dram_tensor("wqq", [E, P], f32r, kind="ExternalInput").ap()
    dmask_d = nc.dram_tensor("dmask", [P, 2, QC], f32r, kind="ExternalInput").ap()
    ident_d = nc.dram_tensor("ident", [P, P], f32r, kind="ExternalInput").ap()
    out_d = nc.dram_tensor("o", [H + 1, NQCH, QC], f32, kind="ExternalOutput").ap()

    with tile.TileContext(nc) as tc:
        with (
            tc.tile_pool(name="consts", bufs=1) as consts,
            tc.tile_pool(name="xin", bufs=4) as xin,
            tc.tile_pool(name="big", bufs=1) as big,
            tc.tile_pool(name="work", bufs=3) as work,
            tc.tile_pool(name="psum", bufs=1, space="PSUM") as psum,
        ):
            # ---- constants ----
            # only wqq gates the first matmul; the rest load after region 0
            wqq_sb = consts.tile([P, ETILES, P], f32r)
            nc.sync.dma_start(wqq_sb[:], wqq_d.rearrange("(ko p) m -> p ko m", p=P))
            ident = consts.tile([P, P], f32r)
            wkv_sb = consts.tile([P, ETILES, P], f32r)
            wvk_sb = consts.tile([P, ETILES, P], f32r)
            dmask_sb = consts.tile([P, 2, QC], f32r)
            ones32 = consts.tile([P, 1], f32)
            nc.vector.memset(ones32[:], 1.0)
            # HAM warmup: ~4-5us of dummy matmuls (no DMA deps) during the
            # initial x-DMA wait flip the PE clock gate to 8/8 before the
            # first real projection matmul lands; cold MMs run at 1.2 GHz
            # instead of 2.4 (the PE_HAM activity window needs ~3.4us of
            # sustained matmul activity to un-throttle).
            wrm = consts.tile([P, QC], f32r)
            nc.vector.memset(wrm[:], 0.0)
            # preload the ACT exp table (~2.7us) during the initial DMA wait
            # so the first real exp doesn't pay it
            actwarm = consts.tile([P, 1], f32r)
            nc.scalar.activation(actwarm[:], ones32[:], AF.Exp, scale=1.0)
            for i in range(10):
                pwarm = psum.tile([P, QC], f32, tag="po", bufs=1,
                                  name=f"pwarm_{i}")
                nc.tensor.matmul(pwarm[:], wrm[:, 0:P], wrm[:],
                                 start=True, stop=True)

            # ---- projections ----
            kv_sb = big.tile([P, TLOC], f32r)   # [K^T;V^T] / [V^T;K^T] by chunk
            q_all = big.tile([P, T], f32r)      # [Q^T; Q^T], all tokens
            v_sb = big.tile([P, NKCH, H + 1], f32r)
            nc.vector.tensor_copy(v_sb[:, :, H:H + 1],
                                  ones32[:, None, :].to_broadcast((P, NKCH, 1)))

            # All x-region DMAs are issued up front (xin has exactly NREG
            # buffers) so the Sync queue never head-of-line blocks a prefetch
            # behind an attention output DMA. Region 0 loads per-etile so the
            # first Q chain starts on etile 0 without waiting for the region.
            xts = []
            for r in range(NREG):
                xt = xin.tile([P, ETILES, RC], f32r, tag="xt")
                xts.append(xt)
                if r == 0:
                    # two dma_starts (each dma_start costs ~650ns of Sync
                    # issue time): the first 4 etiles land early enough to
                    # start the first Q chain, without 8 serialized issues
                    for g in range(2):
                        nc.sync.dma_start(
                            xt[:, 4 * g:4 * (g + 1)],
                            xTa_d[4 * g * P:4 * (g + 1) * P,
                                  r * RC:(r + 1) * RC]
                            .rearrange("(ko p) m -> p ko m", p=P))
                    nc.sync.dma_start(ident[:], ident_d[:])
                    nc.sync.dma_start(
                        wkv_sb[:], wkv_d.rearrange("(ko p) m -> p ko m", p=P))
                    nc.sync.dma_start(
                        wvk_sb[:], wvk_d.rearrange("(ko p) m -> p ko m", p=P))
                    nc.sync.dma_start(dmask_sb[:], dmask_d[:])
                else:
                    # one dma_start per region: amortizes the ~0.6-2us fixed
                    # DMA cost and keeps the SDMA engines continuously fed
                    nc.sync.dma_start(
                        xt[:],
                        xTa_d[:, r * RC:(r + 1) * RC]
                        .rearrange("(ko p) m -> p ko m", p=P))

            def q_half(r, half):
                xt = xts[r]
                pq = psum.tile([P, QC], f32, tag="proj", bufs=2)
                for ko in range(ETILES):
                    nc.tensor.matmul(pq[:], wqq_sb[:, ko],
                                     xt[:, ko, half * QC:(half + 1) * QC],
                                     start=(ko == 0), stop=(ko == ETILES - 1))
                c = r * RC + half * QC
                nc.vector.tensor_copy(q_all[:, c:c + QC], pq[:])

            def kv_region(r):
                xt = xts[r]
                # K,V for the region's even (own-parity) 128-blocks.
                # The PE crashes on strided moving operands, so compact the
                # even blocks into a contiguous tile on DVE first.
                xkv = work.tile([P, ETILES, QC], f32r, tag="xkv", bufs=2)
                for ko in range(ETILES):
                    nc.vector.tensor_copy(
                        xkv[:, ko],
                        xt[:, ko].rearrange("p (u v c) -> p u v c",
                                            v=2, c=KC)[:, :, 0, :])
                # chunks 4r,4r+1 as [K;V]; chunks 4r+2,4r+3 as [V;K]
                pkv = psum.tile([P, QC], f32, tag="proj", bufs=2)
                for ko in range(ETILES):
                    nc.tensor.matmul(pkv[:, 0:2 * KC], wkv_sb[:, ko],
                                     xkv[:, ko, 0:2 * KC],
                                     start=(ko == 0), stop=(ko == ETILES - 1))
                for ko in range(ETILES):
                    nc.tensor.matmul(pkv[:, 2 * KC:QC], wvk_sb[:, ko],
                                     xkv[:, ko, 2 * KC:QC],
                                     start=(ko == 0), stop=(ko == ETILES - 1))
                nc.vector.tensor_copy(kv_sb[:, r * QC:(r + 1) * QC], pkv[:])
                # V^T -> V for the region's 4 local key chunks
                for i in range(4 * r, 4 * r + 4):
                    ptr = psum.tile([P, P], f32r, tag="ptr", bufs=1,
                                    name=f"ptr_{i}")
                    nc.tensor.transpose(ptr[:], kv_sb[:, i * KC:(i + 1) * KC],
                                        ident[:])
                    if i % 4 < 2:   # [K;V]: V is the high half after transpose
                        nc.vector.tensor_copy(v_sb[:, i, 0:H], ptr[:, H:P])
                    else:           # [V;K]: V is the low half
                        nc.vector.tensor_copy(v_sb[:, i, 0:H], ptr[:, 0:H])

            # ---- attention (partial, own-parity keys) ----
            def attn_qchunk(j):
                groups = _attn_groups(j)
                ngrp = j + 1
                po = psum.tile([H + 1, QC], f32, tag="po", bufs=1, name=f"po_{j}")
                qs = q_all[:, j * QC:(j + 1) * QC]

                def s_group(g):
                    ps = psum.tile([P, 2, QC], f32, tag="ps", bufs=2,
                                   name=f"ps_{j}_{g[0]}")
                    for u in range(2):
                        c = g[u]
                        if c % 4 < 2:   # K^T at partitions 0-63
                            nc.tensor.matmul(ps[:, u],
                                             kv_sb[0:H, c * KC:(c + 1) * KC],
                                             qs[0:H, :], start=True, stop=True,
                                             tile_position=(0, 0))
                        else:           # K^T at partitions 64-127
                            nc.tensor.matmul(ps[:, u],
                                             kv_sb[H:P, c * KC:(c + 1) * KC],
                                             qs[H:P, :], start=True, stop=True,
                                             tile_position=(H, 0))
                    return ps

                def exp_group(g, ps, on_dve=False):
                    if on_dve:
                        # Schraudolph 2^z bit trick: non-masked groups only
                        pt16 = work.tile([P, 2, QC], i16, tag="pt", bufs=4,
                                         name=f"pt_{j}_{g[0]}")
                        nc.vector.tensor_scalar(pt16[:], ps[:], SCHRA_A,
                                                SCHRA_B, ALU.mult, ALU.add)
                        return pt16[:].bitcast(f32r)
                    pt = work.tile([P, 2, QC], f32r, tag="pt", bufs=4,
                                   name=f"pt_{j}_{g[0]}")
                    nc.scalar.activation(pt[:], ps[:], AF.Exp,
                                         scale=float(H) ** -0.5)
                    # diagonal chunks get their causal mask (host input data)
                    if g[0] == 2 * j and g[1] == 2 * j + 1:
                        nc.vector.tensor_tensor(pt[:], pt[:], dmask_sb[:],
                                                ALU.mult)
                    else:
                        for u in range(2):
                            if g[u] >= 2 * j:
                                nc.vector.tensor_tensor(
                                    pt[:, u], pt[:, u],
                                    dmask_sb[:, g[u] - 2 * j], ALU.mult)
                    return pt[:]

                def av_group(g, pt, first, last):
                    for u in range(2):
                        nc.tensor.matmul(po[:], v_sb[:, g[u], :], pt[:, u],
                                         start=(first and u == 0),
                                         stop=(last and u == 1))

                # After the last region's projections the kernel tail is
                # paced purely by exp throughput; offload some non-masked
                # groups of the last query chunks to the DVE's fast-exp.
                dve_set = {5: (1, 4), 6: (1, 4, 6), 7: (1, 4, 6)}.get(j, ())

                # software-pipelined emission: S(next) before AV(cur)
                ps = s_group(groups[0])
                pt = exp_group(groups[0], ps, 0 in dve_set)
                for idx in range(1, ngrp):
                    ps2 = s_group(groups[idx])
                    av_group(groups[idx - 1], pt, idx - 1 == 0, False)
                    pt = exp_group(groups[idx], ps2, idx in dve_set)
                av_group(groups[-1], pt, ngrp == 1, True)

                ost = work.tile([H + 1, QC], f32, tag="ost", bufs=2)
                nc.vector.tensor_copy(ost[:], po[:])
                nc.sync.dma_start(out_d[:, j], ost[:])

            # Emission order = engine stream order. Interleave each region's
            # projections with the two query chunks it unlocks; region 0 is
            # further split so j=0's exp (needing only Q-half0 and the
            # region's KV) starts as early as possible.
            q_half(0, 0)
            kv_region(0)
            attn_qchunk(0)
            q_half(0, 1)
            attn_qchunk(1)
            for r in range(1, NREG):
                q_half(r, 0)
                kv_region(r)
                q_half(r, 1)
                attn_qchunk(2 * r)
                attn_qchunk(2 * r + 1)

    nc.compile()
    return nc


def _get_graph():
    if "g" not in _CACHE:
        _CACHE["g"] = _build_graph()
    return _CACHE["g"]


def _perm(p: int) -> np.ndarray:
    """Token column permutation for parity p: own-parity 128-blocks at even
    block positions (identity for p=0, adjacent-block swap for p=1)."""
    blocks = np.arange(T // KC).reshape(-1, 2)
    if p == 1:
        blocks = blocks[:, ::-1]
    return (blocks.reshape(-1)[:, None] * KC + np.arange(KC)[None, :]).reshape(-1)


def _make_masks(p: int) -> np.ndarray:
    """Diagonal-pair masks in permuted column space: column t' of a query
    chunk is global token offset sigma(t'); diag chunks have global key
    offsets 128*p (slot 0) and 128*(p+2) (slot 1) within the chunk."""
    perm = _perm(p)
    sigma = perm[:QC] % QC  # within-chunk token offset pattern (j-independent)
    s = np.arange(P)[:, None]
    m = np.empty((P, 2, QC), np.float32)
    m[:, 0] = (sigma[None, :] - s - KC * p) >= 0
    m[:, 1] = (sigma[None, :] - s - KC * (p + 2)) >= 0
    return m


def _run(x, Wk, Wq, Wv, trace=False):
    from concourse.bass_utils import run_bass_kernel_spmd

    x = np.asarray(x, dtype=np.float32)
    Wk = np.asarray(Wk, dtype=np.float32)
    Wq = np.asarray(Wq, dtype=np.float32)
    Wv = np.asarray(Wv, dtype=np.float32)

    import ml_dtypes
    conv = lambda a: np.asarray(a, dtype=ml_dtypes.bfloat16)
    wkv = conv(np.concatenate([Wk.T, Wv.T], axis=1))
    wvk = conv(np.concatenate([Wv.T, Wk.T], axis=1))
    wqq = conv(np.concatenate([Wq.T, Wq.T], axis=1))
    masks = [conv(_make_masks(0)), conv(_make_masks(1))]
    ident_np = conv(np.eye(P, dtype=np.float32))
    perms = [_perm(0), _perm(1)]

    in_maps = []
    xTb = {}
    for c in range(8):
        b, p = c // 2, c % 2
        if (b, p) not in xTb:
            xTb[(b, p)] = conv(x[b].T[:, perms[p]])
        in_maps.append({"xTa": xTb[(b, p)], "wkv": wkv, "wvk": wvk, "wqq": wqq,
                        "dmask": masks[p], "ident": ident_np})

    nc = _get_graph()
    res = run_bass_kernel_spmd(nc, in_maps, core_ids=list(range(8)), trace=trace)

    out = np.empty((B, T, H), dtype=np.float32)
    for b in range(B):
        o0 = res.results[2 * b]["o"].reshape(H + 1, T)
        o1 = res.results[2 * b + 1]["o"].reshape(H + 1, T)
        # p=1 columns are block-swapped; un-permute before merging
        o1 = o1[:, perms[1]]
        s = o0 + o1
        out[b] = (s[0:H] / s[H:H + 1]).T
    return out, res.exec_time_ns


def kernel(x, Wk, Wq, Wv):
    out, _ = _run(x, Wk, Wq, Wv)
    return out


# revision 27
# speedup vs baseline: 1.0404x; 1.0404x over previous
"""Distributed Trainium2 kernel for a single causal attention head.

Module: k,q,v = x@W{k,q,v}.T ; a = softmax(causal(q@k.T/sqrt(64))) ; out = a@v
Shapes: x (4, 4096, 1024) f32; W* (64, 1024) f32; out (4, 4096, 64) f32.

Sharding (one SPMD launch, 8 cores, no collectives): 4 batches x 2
key-parity halves. Core c: batch b=c//2, parity p=c%2. The 32 key chunks
(128 tokens) of a batch are split by parity (even chunks -> p=0, odd ->
p=1), which makes the causal work *and* the instruction structure
identical on every core: for query chunk j (512 tokens), each core
processes exactly 2j+2 of its local key chunks; its two diagonal mask
tiles arrive as input data. To keep all SBUF addresses SPMD-uniform, the
host hands each core x[b].T with token columns permuted so the core's
own-parity key blocks sit at even 128-block positions (identity for p=0,
adjacent-block swap for p=1).

Per core: project K^T,V^T for own-parity tokens and Q^T for ALL tokens;
V^T -> V by PE transpose (ones column appended -> softmax sums ride
along row 64 of the AV output). The S = K.Q^T matmul contracts only the
64 head dims, so it is ROW-TILED: local key chunks come in two flavors
-- chunks c with c%4<2 are projected with [Wk|Wv] (K^T at partitions
0-63), chunks with c%4>=2 with [Wv|Wk] (K^T at partitions 64-127) --
and Q^T is projected duplicated into both partition halves ([Wq|Wq]
stationary). A (top,bottom) chunk pair then runs as two concurrent
64-contraction matmuls in separate PE row groups (tile_position (0,0) /
(64,0)), halving S cost. P^T = exp(S^T/8) on ACT over the 1024-wide
pair (diagonal chunks multiplied by input masks), then O'^T(65,512) +=
[V|1].T @ P^T per chunk. The partial [O'^T; l] (65, 4096) goes to DRAM;
the host adds the two parity partials per batch, divides by the summed
denominators l, un-permutes and transposes (the standard partial-softmax
combine; no max-subtraction is needed since the logits are O(1) by
construction).

Compute dtype: bf16 matmul operands with f32 PSUM accumulation (~3e-3
rel err). The softmax exp runs on ACT from f32 PSUM.
"""

import numpy as np

B, T, E, H = 4, 4096, 1024, 64
P = 128           # partitions
QC = 512          # query chunk (matmul moving free dim)
KC = 128          # key chunk
ETILES = E // P   # 8 contraction tiles
NKCH = T // KC // 2   # 16 local (parity) key chunks per core
NREG = 4          # 1024-column load/projection regions
NQCH = T // QC    # 8 query chunks
TLOC = T // 2     # 2048 local (own-parity) tokens

_CACHE = {}

COMPUTE = "bf16"


def _attn_groups(j):
    """Chunk pairing for query chunk j: each group is (cA, cB) executed as
    two S matmuls; mixed (top,bottom) groups run concurrently in separate
    PE row groups. Chunk c is 'top' (K^T at partitions 0-63) iff c%4 < 2.
    Groups holding the diagonal (masked) chunks 2j, 2j+1 go at positions
    2-3: late enough that a still-in-flight diagonal region doesn't stall
    the chunk's exp stream, early enough that the DVE mask multiply never
    gates the final AV."""
    tops = [c for c in range(2 * j + 2) if c % 4 < 2]
    bots = [c for c in range(2 * j + 2) if c % 4 >= 2]
    if j == 0:
        return [(0, 1)]
    if j % 2 == 1:  # diag chunks are the last two bottoms
        diag = [(tops[0], 2 * j), (tops[1], 2 * j + 1)]
        rest = list(zip(tops[2:], bots[:-2]))
    else:  # j even: diag chunks are the last two tops; one all-top group
        tr = tops[:-2]
        diag = [(2 * j, bots[0]), (2 * j + 1, bots[1])]
        rest = list(zip(tr[:-2], bots[2:])) + [(tr[-2], tr[-1])]
    return rest[:2] + diag + rest[2:]


def _build_graph():
    import concourse.bass as bass
    import concourse.tile as tile
    from concourse import bacc, mybir
    f32 = mybir.dt.float32
    f32r = mybir.dt.bfloat16
    AF = mybir.ActivationFunctionType
    ALU = mybir.AluOpType
    RC = T // NREG  # 1024 columns per region

    i16 = mybir.dt.int16
    # Schraudolph fast-exp constants: bits16 = round(s * A + B) reinterpreted
    # as bf16 gives exp(s/8) with a +0..6% piecewise-linear sawtooth error
    # that largely cancels in the softmax ratio (numerator and denominator
    # share the same perturbed weights). Used on the DVE for some of the
    # late, non-masked groups, where the kernel tail is otherwise paced by
    # the scalar engine's exp throughput.
    SCHRA_A = 128.0 * float(np.log2(np.e)) / float(H) ** 0.5
    SCHRA_B = 16256.0

    nc = bacc.Bacc("TRN2", target_bir_lowering=False, debug=False, num_devices=8)
    xTa_d = nc.dram_tensor("xTa", [E, T], f32r, kind="ExternalInput").ap()
    wkv_d = nc.dram_tensor("wkv", [E, P], f32r, kind="ExternalInput").ap()
    wvk_d = nc.dram_tensor("wvk", [E, P], f32r, kind="ExternalInput").ap()
    wqq_d = nc.dram_tensor("wqq", [E, P], f32r, kind="ExternalInput").ap()
    dmask_d = nc.dram_tensor("dmask", [P, 2, QC], f32r, kind="ExternalInput").ap()
    ident_d = nc.dram_tensor("ident", [P, P], f32r, kind="ExternalInput").ap()
    out_d = nc.dram_tensor("o", [H + 1, NQCH, QC], f32, kind="ExternalOutput").ap()

    with tile.TileContext(nc) as tc:
        with (
            tc.tile_pool(name="consts", bufs=1) as consts,
            tc.tile_pool(name="xin", bufs=4) as xin,
            tc.tile_pool(name="big", bufs=1) as big,
            tc.tile_pool(name="work", bufs=3) as work,
            tc.tile_pool(name="psum", bufs=1, space="PSUM") as psum,
        ):
            # ---- constants ----
            # only wqq gates the first matmul; the rest load after region 0
            wqq_sb = consts.tile([P, ETILES, P], f32r)
            nc.sync.dma_start(wqq_sb[:], wqq_d.rearrange("(ko p) m -> p ko m", p=P))
            ident = consts.tile([P, P], f32r)
            wkv_sb = consts.tile([P, ETILES, P], f32r)
            wvk_sb = consts.tile([P, ETILES, P], f32r)
            dmask_sb = consts.tile([P, 2, QC], f32r)
            ones32 = consts.tile([P, 1], f32)
            nc.vector.memset(ones32[:], 1.0)
            # HAM warmup: ~4-5us of dummy matmuls (no DMA deps) during the
            # initial x-DMA wait flip the PE clock gate to 8/8 before the
            # first real projection matmul lands; cold MMs run at 1.2 GHz
            # instead of 2.4 (the PE_HAM activity window needs ~3.4us of
            # sustained matmul activity to un-throttle).
            wrm = consts.tile([P, QC], f32r)
            nc.vector.memset(wrm[:], 0.0)
            # preload the ACT exp table (~2.7us) during the initial DMA wait
            # so the first real exp doesn't pay it
            actwarm = consts.tile([P, 1], f32r)
            nc.scalar.activation(actwarm[:], ones32[:], AF.Exp, scale=1.0)
            for i in range(10):
                pwarm = psum.tile([P, QC], f32, tag="po", bufs=1,
                                  name=f"pwarm_{i}")
                nc.tensor.matmul(pwarm[:], wrm[:, 0:P], wrm[:],
                                 start=True, stop=True)

            # ---- projections ----
            kv_sb = big.tile([P, TLOC], f32r)   # [K^T;V^T] / [V^T;K^T] by chunk
            q_all = big.tile([P, T], f32r)      # [Q^T; Q^T], all tokens
            v_sb = big.tile([P, NKCH, H + 1], f32r)
            nc.vector.tensor_copy(v_sb[:, :, H:H + 1],
                                  ones32[:, None, :].to_broadcast((P, NKCH, 1)))

            # All x-region DMAs are issued up front (xin has exactly NREG
            # buffers) so the Sync queue never head-of-line blocks a prefetch
            # behind an attention output DMA. Region 0 loads per-etile so the
            # first Q chain starts on etile 0 without waiting for the region.
            xts = []
            for r in range(NREG):
                xt = xin.tile([P, ETILES, RC], f32r, tag="xt")
                xts.append(xt)
                if r == 0:
                    # two dma_starts (each dma_start costs ~650ns of Sync
                    # issue time): the first 4 etiles land early enough to
                    # start the first Q chain, without 8 serialized issues
                    for g in range(2):
                        nc.sync.dma_start(
                            xt[:, 4 * g:4 * (g + 1)],
                            xTa_d[4 * g * P:4 * (g + 1) * P,
                                  r * RC:(r + 1) * RC]
                            .rearrange("(ko p) m -> p ko m", p=P))
                    nc.sync.dma_start(ident[:], ident_d[:])
                    nc.sync.dma_start(
                        wkv_sb[:], wkv_d.rearrange("(ko p) m -> p ko m", p=P))
                    nc.sync.dma_start(
                        wvk_sb[:], wvk_d.rearrange("(ko p) m -> p ko m", p=P))
                    nc.sync.dma_start(dmask_sb[:], dmask_d[:])
                else:
                    # one dma_start per region: amortizes the ~0.6-2us fixed
                    # DMA cost and keeps the SDMA engines continuously fed
                    nc.sync.dma_start(
                        xt[:],
                        xTa_d[:, r * RC:(r + 1) * RC]
                        .rearrange("(ko p) m -> p ko m", p=P))

            def q_half(r, half):
                xt = xts[r]
                pq = psum.tile([P, QC], f32, tag="proj", bufs=2)
                for ko in range(ETILES):
                    nc.tensor.matmul(pq[:], wqq_sb[:, ko],
                                     xt[:, ko, half * QC:(half + 1) * QC],
                                     start=(ko == 0), stop=(ko == ETILES - 1))
                c = r * RC + half * QC
                nc.vector.tensor_copy(q_all[:, c:c + QC], pq[:])

            def kv_region(r):
                xt = xts[r]
                # K,V for the region's even (own-parity) 128-blocks.
                # The PE crashes on strided moving operands, so compact the
                # even blocks into a contiguous tile on DVE first.
                xkv = work.tile([P, ETILES, QC], f32r, tag="xkv", bufs=2)
                for ko in range(ETILES):
                    nc.vector.tensor_copy(
                        xkv[:, ko],
                        xt[:, ko].rearrange("p (u v c) -> p u v c",
                                            v=2, c=KC)[:, :, 0, :])
                # chunks 4r,4r+1 as [K;V]; chunks 4r+2,4r+3 as [V;K]
                pkv = psum.tile([P, QC], f32, tag="proj", bufs=2)
                for ko in range(ETILES):
                    nc.tensor.matmul(pkv[:, 0:2 * KC], wkv_sb[:, ko],
                                     xkv[:, ko, 0:2 * KC],
                                     start=(ko == 0), stop=(ko == ETILES - 1))
                for ko in range(ETILES):
                    nc.tensor.matmul(pkv[:, 2 * KC:QC], wvk_sb[:, ko],
                                     xkv[:, ko, 2 * KC:QC],
                                     start=(ko == 0), stop=(ko == ETILES - 1))
                nc.vector.tensor_copy(kv_sb[:, r * QC:(r + 1) * QC], pkv[:])
                # V^T -> V for the region's 4 local key chunks
                for i in range(4 * r, 4 * r + 4):
                    ptr = psum.tile([P, P], f32r, tag="ptr", bufs=1,
                                    name=f"ptr_{i}")
                    nc.tensor.transpose(ptr[:], kv_sb[:, i * KC:(i + 1) * KC],
                                        ident[:])
                    if i % 4 < 2:   # [K;V]: V is the high half after transpose
                        nc.vector.tensor_copy(v_sb[:, i, 0:H], ptr[:, H:P])
                    else:           # [V;K]: V is the low half
                        nc.vector.tensor_copy(v_sb[:, i, 0:H], ptr[:, 0:H])

            # ---- attention (partial, own-parity keys) ----
            def attn_qchunk(j):
                groups = _attn_groups(j)
                ngrp = j + 1
                po = psum.tile([H + 1, QC], f32, tag="po", bufs=1, name=f"po_{j}")
                qs = q_all[:, j * QC:(j + 1) * QC]

                HC = QC // 2

                def s_group(g):
                    ps = psum.tile([P, 2, QC], f32, tag="ps", bufs=2,
                                   name=f"ps_{j}_{g[0]}")
                    for u in range(2):
                        c = g[u]
                        # the second diagonal chunk's mask is all-zero in
                        # query columns 0:256 on both parities; stream only
                        # the upper half (exp of the stale lower half is
                        # finite and masked to zero). j=0 streams full: its
                        # ps buffer is PSUM-uninitialized, not stale-finite.
                        lo = HC if (c == 2 * j + 1 and j > 0) else 0
                        if c % 4 < 2:   # K^T at partitions 0-63
                            nc.tensor.matmul(ps[:, u, lo:],
                                             kv_sb[0:H, c * KC:(c + 1) * KC],
                                             qs[0:H, lo:], start=True,
                                             stop=True, tile_position=(0, 0))
                        else:           # K^T at partitions 64-127
                            nc.tensor.matmul(ps[:, u, lo:],
                                             kv_sb[H:P, c * KC:(c + 1) * KC],
                                             qs[H:P, lo:], start=True,
                                             stop=True, tile_position=(H, 0))
                    return ps

                def exp_group(g, ps, on_dve=False):
                    if on_dve:
                        # Schraudolph 2^z bit trick: non-masked groups only
                        pt16 = work.tile([P, 2, QC], i16, tag="pt", bufs=4,
                                         name=f"pt_{j}_{g[0]}")
                        nc.vector.tensor_scalar(pt16[:], ps[:], SCHRA_A,
                                                SCHRA_B, ALU.mult, ALU.add)
                        return pt16[:].bitcast(f32r)
                    pt = work.tile([P, 2, QC], f32r, tag="pt", bufs=4,
                                   name=f"pt_{j}_{g[0]}")
                    nc.scalar.activation(pt[:], ps[:], AF.Exp,
                                         scale=float(H) ** -0.5)
                    # diagonal chunks get their causal mask (host input data)
                    if g[0] == 2 * j and g[1] == 2 * j + 1:
                        nc.vector.tensor_tensor(pt[:], pt[:], dmask_sb[:],
                                                ALU.mult)
                    else:
                        for u in range(2):
                            if g[u] >= 2 * j:
                                nc.vector.tensor_tensor(
                                    pt[:, u], pt[:, u],
                                    dmask_sb[:, g[u] - 2 * j], ALU.mult)
                    return pt[:]

                def av_group(g, pt, first, last):
                    for u in range(2):
                        lo = HC if (g[u] == 2 * j + 1 and j > 0) else 0
                        nc.tensor.matmul(po[:, lo:], v_sb[:, g[u], :],
                                         pt[:, u, lo:],
                                         start=(first and u == 0),
                                         stop=(last and u == 1))

                # After the last region's projections the kernel tail is
                # paced purely by exp throughput; offload some non-masked
                # groups of the last query chunks to the DVE's fast-exp.
                dve_set = {5: (1, 4), 6: (1, 4, 6), 7: (1, 4, 6)}.get(j, ())

                # software-pipelined emission: S(next) before AV(cur)
                ps = s_group(groups[0])
                pt = exp_group(groups[0], ps, 0 in dve_set)
                for idx in range(1, ngrp):
                    ps2 = s_group(groups[idx])
                    av_group(groups[idx - 1], pt, idx - 1 == 0, False)
                    pt = exp_group(groups[idx], ps2, idx in dve_set)
                av_group(groups[-1], pt, ngrp == 1, True)

                ost = work.tile([H + 1, QC], f32, tag="ost", bufs=2)
                nc.vector.tensor_copy(ost[:], po[:])
                nc.sync.dma_start(out_d[:, j], ost[:])

            # Emission order = engine stream order. Interleave each region's
            # projections with the two query chunks it unlocks; region 0 is
            # further split so j=0's exp (needing only Q-half0 and the
            # region's KV) starts as early as possible.
            q_half(0, 0)
            kv_region(0)
            attn_qchunk(0)
            q_half(0, 1)
            attn_qchunk(1)
            for r in range(1, NREG):
                q_half(r, 0)
                kv_region(r)
                q_half(r, 1)
                attn_qchunk(2 * r)
                attn_qchunk(2 * r + 1)

    nc.compile()
    return nc


def _get_graph():
    if "g" not in _CACHE:
        _CACHE["g"] = _build_graph()
    return _CACHE["g"]


def _perm(p: int) -> np.ndarray:
    """Token column permutation for parity p: own-parity 128-blocks at even
    block positions (identity for p=0, adjacent-block swap for p=1)."""
    blocks = np.arange(T // KC).reshape(-1, 2)
    if p == 1:
        blocks = blocks[:, ::-1]
    return (blocks.reshape(-1)[:, None] * KC + np.arange(KC)[None, :]).reshape(-1)


def _make_masks(p: int) -> np.ndarray:
    """Diagonal-pair masks in permuted column space: column t' of a query
    chunk is global token offset sigma(t'); diag chunks have global key
    offsets 128*p (slot 0) and 128*(p+2) (slot 1) within the chunk."""
    perm = _perm(p)
    sigma = perm[:QC] % QC  # within-chunk token offset pattern (j-independent)
    s = np.arange(P)[:, None]
    m = np.empty((P, 2, QC), np.float32)
    m[:, 0] = (sigma[None, :] - s - KC * p) >= 0
    m[:, 1] = (sigma[None, :] - s - KC * (p + 2)) >= 0
    return m


def _run(x, Wk, Wq, Wv, trace=False):
    from concourse.bass_utils import run_bass_kernel_spmd

    x = np.asarray(x, dtype=np.float32)
    Wk = np.asarray(Wk, dtype=np.float32)
    Wq = np.asarray(Wq, dtype=np.float32)
    Wv = np.asarray(Wv, dtype=np.float32)

    import ml_dtypes
    conv = lambda a: np.asarray(a, dtype=ml_dtypes.bfloat16)
    wkv = conv(np.concatenate([Wk.T, Wv.T], axis=1))
    wvk = conv(np.concatenate([Wv.T, Wk.T], axis=1))
    wqq = conv(np.concatenate([Wq.T, Wq.T], axis=1))
    masks = [conv(_make_masks(0)), conv(_make_masks(1))]
    ident_np = conv(np.eye(P, dtype=np.float32))
    perms = [_perm(0), _perm(1)]

    in_maps = []
    xTb = {}
    for c in range(8):
        b, p = c // 2, c % 2
        if (b, p) not in xTb:
            xTb[(b, p)] = conv(x[b].T[:, perms[p]])
        in_maps.append({"xTa": xTb[(b, p)], "wkv": wkv, "wvk": wvk, "wqq": wqq,
                        "dmask": masks[p], "ident": ident_np})

    nc = _get_graph()
    res = run_bass_kernel_spmd(nc, in_maps, core_ids=list(range(8)), trace=trace)

    out = np.empty((B, T, H), dtype=np.float32)
    for b in range(B):
        o0 = res.results[2 * b]["o"].reshape(H + 1, T)
        o1 = res.results[2 * b + 1]["o"].reshape(H + 1, T)
        # p=1 columns are block-swapped; un-permute before merging
        o1 = o1[:, perms[1]]
        s = o0 + o1
        out[b] = (s[0:H] / s[H:H + 1]).T
    return out, res.exec_time_ns


def kernel(x, Wk, Wq, Wv):
    out, _ = _run(x, Wk, Wq, Wv)
    return out
